# revision 11
# baseline (speedup 1.0000x reference)
"""GAE advantage kernel for Trainium2 (Bass/Tile), 8-core SPMD.

Math: v = relu(states @ W1 + b1) @ W2 + b2 ; deltas = gamma*v[1:] + rewards - v[:-1]
      adv[t] = deltas[t] + (gamma*lam) * adv[t+1]   (reverse scan)

Strategy:
  - Data-parallel over T across 8 cores; each core gets a 125k chunk plus a
    512-element halo (decay^512 ~ 1e-16 -> exact to fp32, no collectives).
  - Within a core, timesteps are processed as pairs (G=2): a [128,128] SBUF
    tile holds 128 pairs (2 contiguous rows of states per partition), which
    PE-transposes into [64 even-features; 64 odd-features] x 128 pair columns.
  - MM1 vs blockdiag(W1,W1) gives hidden for both parities; ACT applies
    relu+bias; MM2 uses the hidden chunk as the matmul stationary against a
    [128,2] W2 blockdiag, emitting v for 128 pairs as two PSUM columns.
  - The reverse scan is a blocked linear operator with block B=256 (=1 pair
    column pair): ADV_e = A_ee@D_e + A_eo@D_o (+ carry), where A_* are
    decay-power Toeplitz matrices computed host-side. Cross-block carries use
    q = decay^256 ~ 1.1e-8 so a rank-2 fixup matmul is exact to fp32.
"""

import numpy as np

GAMMA = 0.98
LAM = 0.95
DECAY = np.float32(GAMMA * LAM)
D_STATE = 64
HIDDEN = 64
T = 1_000_000
N_CORES = 8
L = T // N_CORES            # 125000 kept timesteps per core
HALO = 512                  # decay^512 ~ 4e-16 -> below fp32 resolution

# per-core geometry (uniform across cores; SPMD)
N_D = L + HALO              # deltas needed per core (valid count on cores 0-6)
C_BLK = 496                 # number of 256-blocks (pair-tile columns); 496*256=126976
N_PAIRS = C_BLK * 128       # 63488 pairs = 126976 timesteps of states rows
N_ROWS = N_PAIRS * 2        # states rows staged per core
SG = 16                     # pair-tiles per DMA super-group
N_GROUPS = C_BLK // SG      # 31
CV = 492                    # blocks used for deltas/scan (492*256=125952 >= N_D+1)


def _constants():
    r = np.float64(DECAY)
    i = np.arange(128)
    p = np.arange(128)
    d = p[None, :] - i[:, None]
    A_ee = np.where(d >= 0, r ** (2 * d), 0.0).astype(np.float32)
    A_eo = np.where(d >= 0, r ** (2 * d + 1), 0.0).astype(np.float32)
    A_oe = np.where(d > 0, r ** (2 * d - 1), 0.0).astype(np.float32)
    q = r ** 256
    # fixup weights: adv[i (parity n), c] += r^(256-2i-n) * e[c],
    # e[c] = s[c+1] + q*s[c+2]  (q^2 ~ 1e-16, negligible)
    w_e = (r ** (256 - 2 * i)).astype(np.float32)
    w_o = (r ** (255 - 2 * i)).astype(np.float32)
    Wfix_e = w_e[None, :].astype(np.float32)  # [1,128]
    Wfix_o = w_o[None, :].astype(np.float32)
    ident = np.eye(128, dtype=np.float32)
    w1col_e = (r ** (2 * i)).reshape(128, 1).astype(np.float32)
    w1col_o = (r ** (2 * i + 1)).reshape(128, 1).astype(np.float32)
    Sh = np.zeros((128, 128), np.float32)   # lhsT: out[i,:]=V[i+1,:]
    Sh[np.arange(1, 128), np.arange(0, 127)] = 1.0
    E127 = np.zeros((1, 128), np.float32)
    E127[0, 127] = 1.0
    return (A_ee, A_eo, A_oe, Wfix_e, Wfix_o, ident, Sh, E127,
            w1col_e, w1col_o)


def _host_prep(states, rewards, W1, b1, W2, b2):
    """Build per-core input maps (numpy only)."""
    (A_ee, A_eo, A_oe, Wfix_e, Wfix_o, ident, Sh, E127,
     w1col_e, w1col_o) = _constants()
    W1s = np.zeros((128, 128), np.float32)
    W1s[:64, :64] = W1
    W1s[64:, 64:] = W1
    b1s = np.concatenate([b1, b1]).reshape(128, 1).astype(np.float32)
    W2s = np.zeros((128, 2), np.float32)
    W2s[:64, 0] = W2[:, 0]
    W2s[64:, 1] = W2[:, 0]

    gm1b2 = np.float32((GAMMA - 1.0) * float(b2[0]))

    in_maps = []
    for m in range(N_CORES):
        t0 = m * L
        # states rows [t0, t0+N_ROWS), zero-padded past the end
        avail = min(N_ROWS, (T + 1) - t0)
        sc = np.zeros((N_ROWS, D_STATE), np.float32)
        sc[:avail] = states[t0:t0 + avail]
        # valid deltas for this core
        nv = min(N_D, T - t0)
        # rewards + (gamma-1)*b2 on valid slots, 0 on padding; layout: block
        # c, partition p, parity n  ->  t' = 256c + 2p + n
        rx = np.zeros(CV * 256, np.float32)
        rx[:nv] = rewards[t0:t0 + nv] + gm1b2
        rx = rx.reshape(CV, 128, 2)
        mk = np.zeros(CV * 256, np.float32)
        mk[:nv] = 1.0
        mk = mk.reshape(CV, 128, 2)
        in_maps.append({
            "states_c": sc,
            "rext_e": np.ascontiguousarray(rx[:, :, 0].T),   # [128, CV]
            "rext_o": np.ascontiguousarray(rx[:, :, 1].T),
            "mask_e": np.ascontiguousarray(mk[:, :, 0].T),
            "mask_o": np.ascontiguousarray(mk[:, :, 1].T),
            "W1s": W1s, "b1s": b1s, "W2s": W2s,
            "A_ee": np.ascontiguousarray(A_ee.T),
            "A_eo": np.ascontiguousarray(A_eo.T),
            "A_oe": np.ascontiguousarray(A_oe.T),
            "Wfix_e": Wfix_e, "Wfix_o": Wfix_o, "ident": ident,
            "Sh": Sh, "E127": E127, "identr": ident,
            "w1col_e": w1col_e, "w1col_o": w1col_o,
        })
    return in_maps


def _build_bass():
    import concourse.bass as bass
    import concourse.tile as tile
    from concourse import bacc, mybir

    f32 = mybir.dt.float32
    # float32r (single-pass fp22-truncated PE mode) measured 3.1e-4 max abs
    # err on HW vs 2.4e-6 for true fp32; keep fp32 for safety.
    f32r = f32
    nc = bacc.Bacc("TRN2", target_bir_lowering=False, debug=False,
                   num_devices=N_CORES)

    states_c = nc.dram_tensor("states_c", [N_ROWS, D_STATE], f32r,
                              kind="ExternalInput").ap()
    ins = {}
    for spec in [
        ("rext_e", [128, CV]), ("rext_o", [128, CV]),
        ("mask_e", [128, CV]), ("mask_o", [128, CV]),
        ("W1s", [128, 128], "r"), ("b1s", [128, 1]), ("W2s", [128, 2], "r"),
        ("A_ee", [128, 128]), ("A_eo", [128, 128]), ("A_oe", [128, 128]),
        ("Wfix_e", [1, 128]), ("Wfix_o", [1, 128]), ("ident", [128, 128]),
        ("Sh", [128, 128]), ("E127", [1, 128]), ("identr", [128, 128], "r"),
        ("w1col_e", [128, 1]), ("w1col_o", [128, 1]),
    ]:
        name, shape = spec[0], spec[1]
        dt_ = f32r if len(spec) > 2 else f32
        ins[name] = nc.dram_tensor(name, shape, dt_, kind="ExternalInput").ap()
    adv_e = nc.dram_tensor("adv_e", [128, CV], f32, kind="ExternalOutput").ap()
    adv_o = nc.dram_tensor("adv_o", [128, CV], f32, kind="ExternalOutput").ap()

    Relu = mybir.ActivationFunctionType.Relu
    Alu = mybir.AluOpType

    with tile.TileContext(nc) as tc:
        from contextlib import ExitStack
        ctx = ExitStack()
        with ctx:
            cpool = ctx.enter_context(tc.tile_pool(name="consts", bufs=1))
            spool = ctx.enter_context(tc.tile_pool(name="sload", bufs=3))
            big = ctx.enter_context(tc.tile_pool(name="big", bufs=1))
            mlp_ctx = ctx.enter_context(ExitStack())
            tpsum = mlp_ctx.enter_context(
                tc.tile_pool(name="tpsum", bufs=2, space="PSUM"))
            stT_pool = mlp_ctx.enter_context(tc.tile_pool(name="stT", bufs=3))
            hpsum = mlp_ctx.enter_context(
                tc.tile_pool(name="hpsum", bufs=2, space="PSUM"))
            hrel = mlp_ctx.enter_context(tc.tile_pool(name="hrel", bufs=3))
            vpsum = mlp_ctx.enter_context(
                tc.tile_pool(name="vpsum", bufs=2, space="PSUM"))

            # ---- load constants ----
            const = {}
            for name, ap in ins.items():
                t = cpool.tile(list(ap.shape), ap.dtype, tag=name)
                nc.sync.dma_start(out=t[:], in_=ap[:])
                const[name] = t

            # V (value net output), even/odd parity, [128, C_BLK+1]
            V_e = big.tile([128, C_BLK + 1], f32, tag="V_e")
            V_o = big.tile([128, C_BLK + 1], f32, tag="V_o")

            # ---- MLP over all pair-tiles ----
            # states flat view for 3D DMA: offset(s,p,j) = 16384*(SG*g+s) + 128*p + j
            s_flat = states_c.rearrange("a b -> (a b)")
            vseq = 0  # pair-tile counter == block column index
            vps_tile = None
            vps_base = 0
            for g in range(N_GROUPS):
                sload = spool.tile([128, SG * 128], f32r, tag="sload")
                src = s_flat[g * SG * 16384:(g + 1) * SG * 16384]
                src3 = src.rearrange("(s p j) -> p s j", s=SG, p=128, j=128)
                dst3 = sload[:].rearrange("p (s j) -> p s j", s=SG, j=128)
                nc.sync.dma_start(out=dst3, in_=src3)

                for t4 in range(SG // 4):
                    st_ps = tpsum.tile([128, 512], f32r, tag="st_ps")
                    for u in range(4):
                        off = (t4 * 4 + u) * 128
                        nc.tensor.transpose(
                            st_ps[:, u * 128:(u + 1) * 128],
                            sload[:, off:off + 128],
                            const["identr"][:])
                    st_sb = stT_pool.tile([128, 512], f32r, tag="st_sb")
                    if t4 % 2 == 0:
                        nc.vector.tensor_copy(st_sb[:], st_ps[:])
                    else:
                        nc.scalar.copy(st_sb[:], st_ps[:])

                    h_ps = hpsum.tile([128, 512], f32, tag="h_ps")
                    nc.tensor.matmul(h_ps[:], const["W1s"][:], st_sb[:],
                                     start=True, stop=True)
                    h_sb = hrel.tile([128, 512], f32r, tag="h_sb")
                    nc.scalar.activation(h_sb[:], h_ps[:], Relu,
                                         bias=const["b1s"][:], scale=1.0)

                    for c4 in range(4):
                        if vps_tile is None:
                            vps_tile = vpsum.tile([128, 512], f32, tag="vps")
                            vps_base = vseq
                        rel = vseq - vps_base
                        nc.tensor.matmul(
                            vps_tile[:, 2 * rel:2 * rel + 2],
                            h_sb[:, c4 * 128:(c4 + 1) * 128],
                            const["W2s"][:], start=True, stop=True)
                        vseq += 1
                        if vseq - vps_base == 256 or vseq == C_BLK:
                            n = vseq - vps_base
                            # deinterleave pair-major -> V_e / V_o
                            nc.vector.tensor_copy(
                                V_e[:, vps_base:vseq],
                                vps_tile[:].rearrange(
                                    "p (c two) -> p c two", two=2
                                )[:, 0:n, 0])
                            nc.scalar.copy(
                                V_o[:, vps_base:vseq],
                                vps_tile[:].rearrange(
                                    "p (c two) -> p c two", two=2
                                )[:, 0:n, 1])
                            vps_tile = None

            # ---- deltas ----
            mlp_ctx.close()  # release MLP-phase PSUM banks
            late = ctx.enter_context(
                tc.tile_pool(name="late_psum", bufs=1, space="PSUM"))
            apsum = ctx.enter_context(
                tc.tile_pool(name="apsum", bufs=1, space="PSUM"))
            # vps: v[t+1] for odd slots = V_e shifted up one partition; done
            # on PE (partition-offset APs are rejected by the compiler)
            vps_ps = late.tile([128, CV], f32, tag="vps_ps")
            nc.tensor.matmul(vps_ps[:], const["Sh"][:], V_e[:, 0:CV],
                             start=True, stop=False)
            nc.tensor.matmul(vps_ps[:], const["E127"][:], V_e[0:1, 1:CV + 1],
                             start=False, stop=True)

            D_e = big.tile([128, CV], f32, tag="D_e")
            D_o = big.tile([128, CV], f32, tag="D_o")
            t1 = big.tile([128, CV], f32, tag="t1")
            # D_e = (gamma*V_o - V_e)*mask_e + rext_e
            nc.vector.tensor_scalar_mul(t1[:], V_o[:, 0:CV], float(GAMMA))
            nc.vector.tensor_sub(t1[:], t1[:], V_e[:, 0:CV])
            nc.vector.tensor_mul(t1[:], t1[:], const["mask_e"][:])
            nc.vector.tensor_add(D_e[:], t1[:], const["rext_e"][:])
            # D_o = (gamma*vps - V_o)*mask_o + rext_o
            t2 = big.tile([128, CV], f32, tag="t2")
            nc.scalar.mul(t2[:], vps_ps[:], float(GAMMA))
            nc.vector.tensor_sub(t2[:], t2[:], V_o[:, 0:CV])
            nc.vector.tensor_mul(t2[:], t2[:], const["mask_o"][:])
            nc.vector.tensor_add(D_o[:], t2[:], const["rext_o"][:])

            # ---- blocked scan ----
            # s row in its own psum tile: s[c] = sum_tau r^tau * delta(tau, c)
            s_ps = late.tile([1, CV], f32, tag="s_ps")
            nc.tensor.matmul(s_ps[:], const["w1col_e"][:], D_e[:], start=True,
                             stop=False)
            nc.tensor.matmul(s_ps[:], const["w1col_o"][:], D_o[:],
                             start=False, stop=True)
            # carry row: e[c] = s[c+1] + q*s[c+2]
            q256 = float(np.float64(DECAY) ** 256)
            e_row = big.tile([1, CV], f32, tag="e_row")
            e2 = big.tile([1, CV], f32, tag="e2")
            nc.vector.memset(e_row[:], 0.0)
            nc.vector.memset(e2[:], 0.0)
            nc.vector.tensor_copy(e_row[0:1, 0:CV - 1], s_ps[0:1, 1:CV])
            nc.scalar.mul(e2[0:1, 0:CV - 2], s_ps[0:1, 2:CV], q256)
            nc.vector.tensor_add(e_row[:], e_row[:], e2[:])

            adv_e_ps = apsum.tile([128, CV], f32, tag="adv_e_ps")
            adv_o_ps = apsum.tile([128, CV], f32, tag="adv_o_ps")
            nc.tensor.matmul(adv_e_ps[:], const["A_ee"][:], D_e[:], start=True,
                             stop=False)
            nc.tensor.matmul(adv_e_ps[:], const["A_eo"][:], D_o[:],
                             start=False, stop=False)
            nc.tensor.matmul(adv_e_ps[:], const["Wfix_e"][:], e_row[:],
                             start=False, stop=True)
            nc.tensor.matmul(adv_o_ps[:], const["A_oe"][:], D_e[:], start=True,
                             stop=False)
            nc.tensor.matmul(adv_o_ps[:], const["A_ee"][:], D_o[:],
                             start=False, stop=False)
            nc.tensor.matmul(adv_o_ps[:], const["Wfix_o"][:], e_row[:],
                             start=False, stop=True)

            # ---- out ----
            out_e = big.tile([128, CV], f32, tag="out_e")
            out_o = big.tile([128, CV], f32, tag="out_o")
            nc.vector.tensor_copy(out_e[:], adv_e_ps[:])
            nc.scalar.copy(out_o[:], adv_o_ps[:])
            nc.sync.dma_start(out=adv_e[:], in_=out_e[:])
            nc.sync.dma_start(out=adv_o[:], in_=out_o[:])

    nc.compile()
    return nc


_CACHED = {}


def kernel(states, rewards, W1, b1, W2, b2):
    from concourse.bass_utils import run_bass_kernel_spmd

    states = np.asarray(states, np.float32)
    rewards = np.asarray(rewards, np.float32)
    in_maps = _host_prep(states, rewards,
                         np.asarray(W1, np.float32), np.asarray(b1, np.float32),
                         np.asarray(W2, np.float32), np.asarray(b2, np.float32))
    if "nc" not in _CACHED:
        _CACHED["nc"] = _build_bass()
    nc = _CACHED["nc"]
    res = run_bass_kernel_spmd(nc, in_maps, core_ids=list(range(N_CORES)))

    out = np.empty(T, np.float32)
    for m in range(N_CORES):
        ae = res.results[m]["adv_e"]          # [128, CV]
        ao = res.results[m]["adv_o"]
        blk = np.stack([ae.T, ao.T], axis=-1)  # [CV, 128, 2] -> t'=256c+2p+n
        out[m * L:(m + 1) * L] = blk.reshape(-1)[:L]
    return out


# revision 17
# speedup vs baseline: 1.1779x; 1.1779x over previous
"""GAE advantage kernel for Trainium2 (Bass/Tile), 8-core SPMD.

Math: v = relu(states @ W1 + b1) @ W2 + b2 ; deltas = gamma*v[1:] + rewards - v[:-1]
      adv[t] = deltas[t] + (gamma*lam) * adv[t+1]   (reverse scan)

Strategy:
  - Data-parallel over T across 8 cores; each core gets a 125k chunk plus a
    512-element halo (decay^512 ~ 1e-16 -> exact to fp32, no collectives).
  - Within a core, timesteps are processed as pairs (G=2): a [128,128] SBUF
    tile holds 128 pairs (2 contiguous rows of states per partition), which
    PE-transposes into [64 even-features; 64 odd-features] x 128 pair columns.
  - MM1 vs blockdiag(W1,W1) gives hidden for both parities; ACT applies
    relu+bias; MM2 uses the hidden chunk as the matmul stationary against a
    [128,2] W2 blockdiag, emitting v for 128 pairs as two PSUM columns.
  - The reverse scan is a blocked linear operator with block B=256 (=1 pair
    column pair): ADV_e = A_ee@D_e + A_eo@D_o (+ carry), where A_* are
    decay-power Toeplitz matrices computed host-side. Cross-block carries use
    q = decay^256 ~ 1.1e-8 so a rank-2 fixup matmul is exact to fp32.
"""

import numpy as np
import os

KN_SPOOL = int(os.environ.get("KN_SPOOL", "4"))
KN_TP = int(os.environ.get("KN_TP", "4"))
KN_ST = int(os.environ.get("KN_ST", "3"))
KN_HP = int(os.environ.get("KN_HP", "2"))
KN_HREL = int(os.environ.get("KN_HREL", "3"))
KN_RELU_DVE = int(os.environ.get("KN_RELU_DVE", "0"))
KN_SG = int(os.environ.get("KN_SG", "8"))
KN_VP = int(os.environ.get("KN_VP", "2"))

GAMMA = 0.98
LAM = 0.95
DECAY = np.float32(GAMMA * LAM)
D_STATE = 64
HIDDEN = 64
T = 1_000_000
N_CORES = 8
L = T // N_CORES            # 125000 kept timesteps per core
HALO = 512                  # decay^512 ~ 4e-16 -> below fp32 resolution

# per-core geometry (uniform across cores; SPMD)
N_D = L + HALO              # deltas needed per core (valid count on cores 0-6)
C_BLK = 496                 # number of 256-blocks (pair-tile columns); 496*256=126976
N_PAIRS = C_BLK * 128       # 63488 pairs = 126976 timesteps of states rows
N_ROWS = N_PAIRS * 2        # states rows staged per core
SG = KN_SG                  # pair-tiles per DMA super-group
N_GROUPS = C_BLK // SG      # 31
CV = 492                    # blocks used for deltas/scan (492*256=125952 >= N_D+1)


def _constants():
    r = np.float64(DECAY)
    i = np.arange(128)
    p = np.arange(128)
    d = p[None, :] - i[:, None]
    A_ee = np.where(d >= 0, r ** (2 * d), 0.0).astype(np.float32)
    A_eo = np.where(d >= 0, r ** (2 * d + 1), 0.0).astype(np.float32)
    A_oe = np.where(d > 0, r ** (2 * d - 1), 0.0).astype(np.float32)
    q = r ** 256
    # fixup weights: adv[i (parity n), c] += r^(256-2i-n) * e[c],
    # e[c] = s[c+1] + q*s[c+2]  (q^2 ~ 1e-16, negligible)
    w_e = (r ** (256 - 2 * i)).astype(np.float32)
    w_o = (r ** (255 - 2 * i)).astype(np.float32)
    Wfix_e = w_e[None, :].astype(np.float32)  # [1,128]
    Wfix_o = w_o[None, :].astype(np.float32)
    ident = np.eye(128, dtype=np.float32)
    w1col_e = (r ** (2 * i)).reshape(128, 1).astype(np.float32)
    w1col_o = (r ** (2 * i + 1)).reshape(128, 1).astype(np.float32)
    Sh = np.zeros((128, 128), np.float32)   # lhsT: out[i,:]=V[i+1,:]
    Sh[np.arange(1, 128), np.arange(0, 127)] = 1.0
    E127 = np.zeros((1, 128), np.float32)
    E127[0, 127] = 1.0
    return (A_ee, A_eo, A_oe, Wfix_e, Wfix_o, ident, Sh, E127,
            w1col_e, w1col_o)


def _host_prep(states, rewards, W1, b1, W2, b2):
    """Build per-core input maps (numpy only)."""
    (A_ee, A_eo, A_oe, Wfix_e, Wfix_o, ident, Sh, E127,
     w1col_e, w1col_o) = _constants()
    W1s = np.zeros((128, 128), np.float32)
    W1s[:64, :64] = W1
    W1s[64:, 64:] = W1
    b1s = np.concatenate([b1, b1]).reshape(128, 1).astype(np.float32)
    W2s = np.zeros((128, 2), np.float32)
    W2s[:64, 0] = W2[:, 0]
    W2s[64:, 1] = W2[:, 0]

    gm1b2 = np.float32((GAMMA - 1.0) * float(b2[0]))

    in_maps = []
    for m in range(N_CORES):
        t0 = m * L
        # states rows [t0, t0+N_ROWS), zero-padded past the end
        avail = min(N_ROWS, (T + 1) - t0)
        sc = np.zeros((N_ROWS, D_STATE), np.float32)
        sc[:avail] = states[t0:t0 + avail]
        # valid deltas for this core
        nv = min(N_D, T - t0)
        # rewards + (gamma-1)*b2 on valid slots, 0 on padding; layout: block
        # c, partition p, parity n  ->  t' = 256c + 2p + n
        rx = np.zeros(CV * 256, np.float32)
        rx[:nv] = rewards[t0:t0 + nv] + gm1b2
        rx = rx.reshape(CV, 128, 2)
        mk = np.zeros(CV * 256, np.float32)
        mk[:nv] = 1.0
        mk = mk.reshape(CV, 128, 2)
        in_maps.append({
            "states_c": sc,
            "rext_e": np.ascontiguousarray(rx[:, :, 0].T),   # [128, CV]
            "rext_o": np.ascontiguousarray(rx[:, :, 1].T),
            "mask_e": np.ascontiguousarray(mk[:, :, 0].T),
            "mask_o": np.ascontiguousarray(mk[:, :, 1].T),
            "W1s": W1s, "b1s": b1s, "W2s": W2s,
            "A_ee": np.ascontiguousarray(A_ee.T),
            "A_eo": np.ascontiguousarray(A_eo.T),
            "A_oe": np.ascontiguousarray(A_oe.T),
            "Wfix_e": Wfix_e, "Wfix_o": Wfix_o, "ident": ident,
            "Sh": Sh, "E127": E127, "identr": ident,
            "w1col_e": w1col_e, "w1col_o": w1col_o,
        })
    return in_maps


def _build_bass():
    import concourse.bass as bass
    import concourse.tile as tile
    from concourse import bacc, mybir

    f32 = mybir.dt.float32
    # float32r (single-pass fp22-truncated PE mode) measured 3.1e-4 max abs
    # err on HW vs 2.4e-6 for true fp32; keep fp32 for safety.
    f32r = f32
    nc = bacc.Bacc("TRN2", target_bir_lowering=False, debug=False,
                   num_devices=N_CORES)

    states_c = nc.dram_tensor("states_c", [N_ROWS, D_STATE], f32r,
                              kind="ExternalInput").ap()
    ins = {}
    for spec in [
        ("rext_e", [128, CV]), ("rext_o", [128, CV]),
        ("mask_e", [128, CV]), ("mask_o", [128, CV]),
        ("W1s", [128, 128], "r"), ("b1s", [128, 1]), ("W2s", [128, 2], "r"),
        ("A_ee", [128, 128]), ("A_eo", [128, 128]), ("A_oe", [128, 128]),
        ("Wfix_e", [1, 128]), ("Wfix_o", [1, 128]), ("ident", [128, 128]),
        ("Sh", [128, 128]), ("E127", [1, 128]), ("identr", [128, 128], "r"),
        ("w1col_e", [128, 1]), ("w1col_o", [128, 1]),
    ]:
        name, shape = spec[0], spec[1]
        dt_ = f32r if len(spec) > 2 else f32
        ins[name] = nc.dram_tensor(name, shape, dt_, kind="ExternalInput").ap()
    adv_e = nc.dram_tensor("adv_e", [128, CV], f32, kind="ExternalOutput").ap()
    adv_o = nc.dram_tensor("adv_o", [128, CV], f32, kind="ExternalOutput").ap()

    Relu = mybir.ActivationFunctionType.Relu
    Alu = mybir.AluOpType

    with tile.TileContext(nc) as tc:
        from contextlib import ExitStack
        ctx = ExitStack()
        with ctx:
            cpool = ctx.enter_context(tc.tile_pool(name="consts", bufs=1))
            spool = ctx.enter_context(tc.tile_pool(name="sload", bufs=KN_SPOOL))
            big = ctx.enter_context(tc.tile_pool(name="big", bufs=1))
            mlp_ctx = ctx.enter_context(ExitStack())
            tpsum = mlp_ctx.enter_context(
                tc.tile_pool(name="tpsum", bufs=KN_TP, space="PSUM"))
            stT_pool = mlp_ctx.enter_context(tc.tile_pool(name="stT", bufs=KN_ST))
            hpsum = mlp_ctx.enter_context(
                tc.tile_pool(name="hpsum", bufs=KN_HP, space="PSUM"))
            hrel = mlp_ctx.enter_context(tc.tile_pool(name="hrel", bufs=KN_HREL))
            vpsum = mlp_ctx.enter_context(
                tc.tile_pool(name="vpsum", bufs=KN_VP, space="PSUM"))

            # ---- load constants ----
            const = {}
            for name, ap in ins.items():
                t = cpool.tile(list(ap.shape), ap.dtype, tag=name)
                nc.sync.dma_start(out=t[:], in_=ap[:])
                const[name] = t

            # V (value net output), even/odd parity, [128, C_BLK+1]
            V_e = big.tile([128, C_BLK + 1], f32, tag="V_e")
            V_o = big.tile([128, C_BLK + 1], f32, tag="V_o")

            # ---- MLP over all pair-tiles ----
            # states flat view for 3D DMA: offset(s,p,j) = 16384*(SG*g+s) + 128*p + j
            s_flat = states_c.rearrange("a b -> (a b)")
            vseq = 0  # pair-tile counter == block column index
            vps_tile = None
            vps_base = 0
            for g in range(N_GROUPS):
                sload = spool.tile([128, SG * 128], f32r, tag="sload")
                src = s_flat[g * SG * 16384:(g + 1) * SG * 16384]
                src3 = src.rearrange("(s p j) -> p s j", s=SG, p=128, j=128)
                dst3 = sload[:].rearrange("p (s j) -> p s j", s=SG, j=128)
                nc.sync.dma_start(out=dst3, in_=src3)

                for t4 in range(SG // 4):
                    st_ps = tpsum.tile([128, 512], f32r, tag="st_ps")
                    for u in range(4):
                        off = (t4 * 4 + u) * 128
                        nc.tensor.transpose(
                            st_ps[:, u * 128:(u + 1) * 128],
                            sload[:, off:off + 128],
                            const["identr"][:])
                    st_sb = stT_pool.tile([128, 512], f32r, tag="st_sb")
                    if t4 % 2 == 0:
                        nc.vector.tensor_copy(st_sb[:], st_ps[:])
                    else:
                        nc.scalar.copy(st_sb[:], st_ps[:])

                    h_ps = hpsum.tile([128, 512], f32, tag="h_ps")
                    nc.tensor.matmul(h_ps[:], const["W1s"][:], st_sb[:],
                                     start=True, stop=True)
                    h_sb = hrel.tile([128, 512], f32r, tag="h_sb")
                    if KN_RELU_DVE and (t4 % 2 == 1):
                        nc.vector.tensor_scalar(
                            h_sb[:], h_ps[:], const["b1s"][:], 0.0,
                            op0=Alu.add, op1=Alu.max)
                    else:
                        nc.scalar.activation(h_sb[:], h_ps[:], Relu,
                                             bias=const["b1s"][:], scale=1.0)

                    for c4 in range(4):
                        if vps_tile is None:
                            vps_tile = vpsum.tile([128, 512], f32, tag="vps")
                            vps_base = vseq
                        rel = vseq - vps_base
                        nc.tensor.matmul(
                            vps_tile[:, 2 * rel:2 * rel + 2],
                            h_sb[:, c4 * 128:(c4 + 1) * 128],
                            const["W2s"][:], start=True, stop=True)
                        vseq += 1
                        if vseq - vps_base == 256 or vseq == C_BLK:
                            n = vseq - vps_base
                            # deinterleave pair-major -> V_e / V_o
                            nc.vector.tensor_copy(
                                V_e[:, vps_base:vseq],
                                vps_tile[:].rearrange(
                                    "p (c two) -> p c two", two=2
                                )[:, 0:n, 0])
                            nc.scalar.copy(
                                V_o[:, vps_base:vseq],
                                vps_tile[:].rearrange(
                                    "p (c two) -> p c two", two=2
                                )[:, 0:n, 1])
                            vps_tile = None

            # ---- deltas ----
            mlp_ctx.close()  # release MLP-phase PSUM banks
            late = ctx.enter_context(
                tc.tile_pool(name="late_psum", bufs=1, space="PSUM"))
            apsum = ctx.enter_context(
                tc.tile_pool(name="apsum", bufs=1, space="PSUM"))
            # vps: v[t+1] for odd slots = V_e shifted up one partition; done
            # on PE (partition-offset APs are rejected by the compiler)
            vps_ps = late.tile([128, CV], f32, tag="vps_ps")
            nc.tensor.matmul(vps_ps[:], const["Sh"][:], V_e[:, 0:CV],
                             start=True, stop=False)
            nc.tensor.matmul(vps_ps[:], const["E127"][:], V_e[0:1, 1:CV + 1],
                             start=False, stop=True)

            D_e = big.tile([128, CV], f32, tag="D_e")
            D_o = big.tile([128, CV], f32, tag="D_o")
            t1 = big.tile([128, CV], f32, tag="t1")
            # D_e = (gamma*V_o - V_e)*mask_e + rext_e
            nc.vector.tensor_scalar_mul(t1[:], V_o[:, 0:CV], float(GAMMA))
            nc.vector.tensor_sub(t1[:], t1[:], V_e[:, 0:CV])
            nc.vector.tensor_mul(t1[:], t1[:], const["mask_e"][:])
            nc.vector.tensor_add(D_e[:], t1[:], const["rext_e"][:])
            # D_o = (gamma*vps - V_o)*mask_o + rext_o
            t2 = big.tile([128, CV], f32, tag="t2")
            nc.scalar.mul(t2[:], vps_ps[:], float(GAMMA))
            nc.vector.tensor_sub(t2[:], t2[:], V_o[:, 0:CV])
            nc.vector.tensor_mul(t2[:], t2[:], const["mask_o"][:])
            nc.vector.tensor_add(D_o[:], t2[:], const["rext_o"][:])

            # ---- blocked scan ----
            # s row in its own psum tile: s[c] = sum_tau r^tau * delta(tau, c)
            s_ps = late.tile([1, CV], f32, tag="s_ps")
            nc.tensor.matmul(s_ps[:], const["w1col_e"][:], D_e[:], start=True,
                             stop=False)
            nc.tensor.matmul(s_ps[:], const["w1col_o"][:], D_o[:],
                             start=False, stop=True)
            # carry row: e[c] = s[c+1] + q*s[c+2]
            q256 = float(np.float64(DECAY) ** 256)
            e_row = big.tile([1, CV], f32, tag="e_row")
            e2 = big.tile([1, CV], f32, tag="e2")
            nc.vector.memset(e_row[:], 0.0)
            nc.vector.memset(e2[:], 0.0)
            nc.vector.tensor_copy(e_row[0:1, 0:CV - 1], s_ps[0:1, 1:CV])
            nc.scalar.mul(e2[0:1, 0:CV - 2], s_ps[0:1, 2:CV], q256)
            nc.vector.tensor_add(e_row[:], e_row[:], e2[:])

            adv_e_ps = apsum.tile([128, CV], f32, tag="adv_e_ps")
            adv_o_ps = apsum.tile([128, CV], f32, tag="adv_o_ps")
            nc.tensor.matmul(adv_e_ps[:], const["A_ee"][:], D_e[:], start=True,
                             stop=False)
            nc.tensor.matmul(adv_e_ps[:], const["A_eo"][:], D_o[:],
                             start=False, stop=False)
            nc.tensor.matmul(adv_e_ps[:], const["Wfix_e"][:], e_row[:],
                             start=False, stop=True)
            nc.tensor.matmul(adv_o_ps[:], const["A_oe"][:], D_e[:], start=True,
                             stop=False)
            nc.tensor.matmul(adv_o_ps[:], const["A_ee"][:], D_o[:],
                             start=False, stop=False)
            nc.tensor.matmul(adv_o_ps[:], const["Wfix_o"][:], e_row[:],
                             start=False, stop=True)

            # ---- out ----
            out_e = big.tile([128, CV], f32, tag="out_e")
            out_o = big.tile([128, CV], f32, tag="out_o")
            nc.vector.tensor_copy(out_e[:], adv_e_ps[:])
            nc.scalar.copy(out_o[:], adv_o_ps[:])
            nc.sync.dma_start(out=adv_e[:], in_=out_e[:])
            nc.sync.dma_start(out=adv_o[:], in_=out_o[:])

    nc.compile()
    return nc


_CACHED = {}


def kernel(states, rewards, W1, b1, W2, b2):
    from concourse.bass_utils import run_bass_kernel_spmd

    states = np.asarray(states, np.float32)
    rewards = np.asarray(rewards, np.float32)
    in_maps = _host_prep(states, rewards,
                         np.asarray(W1, np.float32), np.asarray(b1, np.float32),
                         np.asarray(W2, np.float32), np.asarray(b2, np.float32))
    if "nc" not in _CACHED:
        _CACHED["nc"] = _build_bass()
    nc = _CACHED["nc"]
    res = run_bass_kernel_spmd(nc, in_maps, core_ids=list(range(N_CORES)))

    out = np.empty(T, np.float32)
    for m in range(N_CORES):
        ae = res.results[m]["adv_e"]          # [128, CV]
        ao = res.results[m]["adv_o"]
        blk = np.stack([ae.T, ao.T], axis=-1)  # [CV, 128, 2] -> t'=256c+2p+n
        out[m * L:(m + 1) * L] = blk.reshape(-1)[:L]
    return out
